# revision 19
# baseline (speedup 1.0000x reference)
"""Trainium2 Bass kernel for nn_ConvAttentionHybrid.

Math: the reference broadcasts the conv-sigmoid output f[s] along the embed
dim E, so q/k/v are affine (rank-1) in f.  The softmax logits collapse to
    l[s,t] = g[s]*f[t] + (terms constant in t),   g[s] = 0.5*(A*f[s] + C)
with A = rowsum(Wq).rowsum(Wk), C = bq.rowsum(Wk).  With h = f - 1/2:
    m(s) = Num(s)/Den(s)
    Den(s) = sum_n g^n/n! * W_n,          W_n = sum_t h_t^n
    Num(s) = sum_n g^n/n! * (W_{n+1} + W_n/2)
(the common e^{g/2} factor cancels in the ratio), and
    result = sv_sum*sum_s m(s)/(4*S) + bv_sum/4.
|g| <= ~1.1 and |h| <= 1/2 here, so 7 Taylor terms are exact to ~3e-6,
far below the 2e-2 gate.  Each core computes f and the moments fully
(cheap) and evaluates m(s) for its own 2048-element s-chunk, fed by a
host-sliced [128,34] window of `data` (so no one-hot select matmul);
the host sums the 8 partial outputs.

Perf structure vs the first version:
  - all params packed into ONE [4,20] DMA; A/C/sv_sum/bv_sum come from
    one tiny PE matmul; conv weights broadcast to 128 partitions via PE
    (no 128-line broadcast DMAs in front of the data DMAs)
  - dataA on sync queue, dataB issued from the tensor engine queue so
    the two 66KB loads run on different rings
  - conv split vector/gpsimd (2+2 shifted MACs), sigmoid accumulates W_1
  - moments: W_2/W_4 as Square-activations with accum on scalar, W_3/5/6/7
    as fused tensor_tensor_reduce on vector, running concurrently
  - Horner: Den chain on gpsimd, Num chain on vector, 7 coeffs each
"""

import math
from contextlib import ExitStack

import numpy as np

import concourse.bass as bass
import concourse.tile as tile
from concourse import bacc, mybir
from concourse.bass_utils import run_bass_kernel_spmd

AF = mybir.ActivationFunctionType
OP = mybir.AluOpType
AX = mybir.AxisListType
F32 = mybir.dt.float32

NCORES = 8
NCOEF = 4             # Taylor coefficients n = 0..NCOEF-1
NMOM = NCOEF + 1      # moments W_0 .. W_NCOEF
JS = 16               # s-chunk columns per core (128*16 = 2048 s per core)
S_TOTAL = 16384


def _emit(ctx: ExitStack, tc: "tile.TileContext", d):
    nc = tc.nc
    pool = ctx.enter_context(tc.tile_pool(name="main", bufs=1))
    psum = ctx.enter_context(tc.tile_pool(name="ps", bufs=1, space="PSUM"))

    def T(name, shape):
        return pool.tile(shape, F32, tag=name, name=name)

    def P(name, shape):
        return psum.tile(shape, F32, tag=name, name=name)

    # ---------------- DMAs: dataA alone on sync (earliest issue) ------------
    dataA = T("dataA", [128, 129])
    nc.sync.dma_start(out=dataA[:, :], in_=d["data"].ap()[0:128, :])
    dataB = T("dataB", [128, 129])
    nc.scalar.dma_start(out=dataB[:, 0:65], in_=d["data"].ap()[1:129, 0:65])
    pk = T("pk", [4, 21])
    nc.gpsimd.dma_start(out=pk[:, :], in_=d["EP"].ap()[0:4, 16:37])
    nc.gpsimd.dma_start(out=dataB[:, 65:129], in_=d["data"].ap()[1:129, 65:129])
    e_sb = T("e_sb", [128, JS])
    nc.gpsimd.dma_start(out=e_sb[:, :], in_=d["EP"].ap()[:, 0:16])

    # ---------------- early constants (vector + gpsimd, in DMA shadow) -----
    ones4x128 = T("ones4x128", [4, 128])
    halfrow = T("halfrow", [1, 128])
    ones4c = T("ones4c", [4, 1])
    wacc = T("wacc", [128, NMOM])
    nc.vector.memset(ones4x128[:, :], 1.0)
    nc.vector.memset(halfrow[:, :], 0.5)
    nc.vector.memset(ones4c[:, :], 1.0)
    nc.vector.memset(wacc[:, NMOM - 1:NMOM], 128.0)    # W_0 partial (=128*128)

    ones128 = T("ones128", [128, 128])
    invfT = T("invfT", [128, NCOEF])
    invfT2 = T("invfT2", [128, NCOEF])
    nc.gpsimd.memset(ones128[:, :], 1.0)
    for k in range(NCOEF):
        n = NCOEF - 1 - k
        nc.gpsimd.memset(invfT[:, k:k + 1], 1.0 / (math.factorial(n) * 2.0 ** n))
        nc.gpsimd.memset(invfT2[:, k:k + 1], 1.0 / (math.factorial(n) * 2.0 ** (n + 1)))

    # dummy activations: trigger act table loads while DMAs are in flight
    dums = T("dums", [4, 1])
    nc.scalar.activation(dums[:, :], ones4x128[0:4, 0:1], AF.Sigmoid, bias=0.0, scale=1.0)
    nc.scalar.activation(dums[:, :], ones4x128[0:4, 0:1], AF.Tanh, bias=0.0, scale=1.0)
    nc.scalar.activation(dums[:, :], ones4x128[0:4, 0:1], AF.Square, bias=0.0, scale=1.0)

    # ---------------- PE: param matmuls (only need pk) ----------------------
    # conv-weight broadcast: wb[128, 0:4] = w00,w01,w10,w11; col 4 = conv_b
    wb_ps = P("wb_ps", [128, 6])
    nc.tensor.matmul(wb_ps[:, :], ones4x128[:, :], pk[:, 15:21], start=True, stop=True)

    # vector: wb to SBUF; row-sum columns on partitions 0:4
    wbs = T("wbs", [128, 6])
    nc.vector.tensor_copy(wbs[:, :], wb_ps[:, :])
    akc = T("akc", [4, 1])
    rhs4 = T("rhs4", [4, 4])
    nc.vector.reduce_sum(akc[:, :], pk[:, 0:4], axis=AX.X)
    nc.vector.reduce_sum(rhs4[:, 0:1], pk[:, 5:9], axis=AX.X)
    nc.vector.tensor_copy(rhs4[:, 1:2], pk[:, 9:10])
    nc.vector.reduce_sum(rhs4[:, 2:3], pk[:, 10:14], axis=AX.X)
    nc.vector.tensor_copy(rhs4[:, 3:4], pk[:, 14:15])

    # PE: [A, C] = ak^T @ [aq|bq],  [sv, bvs] = ones^T @ [svc|bvc]
    ac_ps = P("ac_ps", [1, 2])
    nc.tensor.matmul(ac_ps[:, :], akc[:, :], rhs4[:, 0:2], start=True, stop=True)
    sb_ps = P("sb_ps", [1, 2])
    nc.tensor.matmul(sb_ps[:, :], ones4c[:, :], rhs4[:, 2:4], start=True, stop=True)

    # ---------------- conv + sigmoid -> f [128,128] ------------------------
    # u1 on scalar (Copy with per-partition scale), u2..u4 chained on vector
    u1 = T("u1", [128, 128]); u2 = T("u2", [128, 128])
    u3 = T("u3", [128, 128]); u4 = T("u4", [128, 128])
    h = T("h", [128, 128]); h2 = T("h2", [128, 128])
    h3 = T("h3", [128, 128])
    junk = T("junk", [128, 128])
    with tc.high_priority():
        nc.scalar.activation(u1[:, :], dataA[:, 0:128], AF.Copy, bias=0.0, scale=wbs[:, 0:1])
        nc.vector.scalar_tensor_tensor(u2[:, :], dataA[:, 1:129], wbs[:, 1:2], u1[:, :], OP.mult, OP.add)
        nc.vector.scalar_tensor_tensor(u3[:, :], dataB[:, 0:128], wbs[:, 2:3], u2[:, :], OP.mult, OP.add)
        nc.vector.scalar_tensor_tensor(u4[:, :], dataB[:, 1:129], wbs[:, 3:4], u3[:, :], OP.mult, OP.add)
        # t = tanh(0.5*conv + 0.5*cb) = 2h  (W_n = W'_n/2^n folded into
        # invfT/invfT2); W'_1 on vector so scalar goes straight to squares
        nc.scalar.activation(h[:, :], u4[:, :], AF.Tanh, bias=wbs[:, 4:5], scale=0.5)

        # ---------------- moments W'_n = sum t^n ----------------------------
        # wacc col j holds the per-partition partial of W'_{NMOM-1-j}
        nc.scalar.activation(h2[:, :], h[:, :], AF.Square, bias=0.0, scale=1.0,
                             accum_out=wacc[:, NMOM - 3:NMOM - 2])
        nc.scalar.activation(junk[:, :], h2[:, :], AF.Square, bias=0.0, scale=1.0,
                             accum_out=wacc[:, NMOM - 5:NMOM - 4])
        nc.vector.reduce_sum(wacc[:, NMOM - 2:NMOM - 1], h[:, :], axis=AX.X)
        nc.vector.tensor_mul(h3[:, :], h2[:, :], h[:, :])
        nc.vector.reduce_sum(wacc[:, NMOM - 4:NMOM - 3], h3[:, :], axis=AX.X)

    # ---------------- per-core chunk: one-hot row select of conv(u4) --------
    # sel[m,j] = u4[16c+j, m] (transposed chunk; sum over s is order-free)
    sel_ps = P("sel_ps", [128, JS])
    nc.tensor.matmul(sel_ps[:, :], u4[:, :], e_sb[:, :], start=True, stop=True)

    # scalar: chunk sigmoid + g = halfA*fc + halfC (needs pbc from PE chain)
    bc_ps = P("bc_ps", [128, 2])
    ac_sb = T("ac_sb", [1, 2])
    sb_sb = T("sb_sb", [1, 2])
    nc.vector.tensor_copy(ac_sb[:, :], ac_ps[:, :])
    nc.vector.tensor_copy(sb_sb[:, :], sb_ps[:, :])
    nc.tensor.matmul(bc_ps[:, :], halfrow[:, :], ac_sb[0:1, 0:2], start=True, stop=True)
    pbc = T("pbc", [128, 2])
    nc.vector.tensor_copy(pbc[:, :], bc_ps[:, :])
    bvt = T("bvt", [1, 1])
    nc.vector.tensor_scalar_mul(bvt[:, :], sb_sb[0:1, 1:2], 1.0 / (4.0 * NCORES))
    svs = T("svs", [1, 1])
    nc.vector.tensor_scalar_mul(svs[:, :], sb_sb[0:1, 0:1], 1.0 / (4.0 * S_TOTAL))

    fc = T("fc", [128, JS])
    ga = T("ga", [128, JS])
    g = T("g", [128, JS])
    nc.scalar.activation(fc[:, :], sel_ps[:, :], AF.Sigmoid, bias=wbs[:, 5:6], scale=1.0)
    nc.vector.scalar_tensor_tensor(ga[:, :], fc[:, :], pbc[:, 0:1], fc[:, :], OP.mult, OP.bypass)
    nc.vector.scalar_tensor_tensor(g[:, :], ga[:, :], pbc[:, 1:2], ga[:, :], OP.add, OP.bypass)

    # ---------------- W totals broadcast to all partitions ------------------
    wrow_ps = P("wrow_ps", [128, NMOM])
    nc.tensor.matmul(wrow_ps[:, :], ones128[:, :], wacc[:, 0:NMOM], start=True, stop=True)

    # ---------------- Taylor coefficients (Horner order, all partitions) ----
    # cden[k] = W_{N-1-k}/(N-1-k)!,  cnum[k] = 0.5*cden[k] + W_{N-k}/(N-1-k)!
    cden = T("cden", [128, NCOEF])
    cdsh = T("cdsh", [128, NCOEF])
    cnum = T("cnum", [128, NCOEF])
    nc.vector.tensor_mul(cden[:, :], wrow_ps[:, 1:NMOM], invfT[:, :])
    nc.vector.tensor_mul(cdsh[:, :], wrow_ps[:, 0:NCOEF], invfT2[:, :])
    nc.vector.scalar_tensor_tensor(cnum[:, :], cden[:, :], 0.5, cdsh[:, :], OP.mult, OP.add)

    # ---------------- fused Den/Num Horner on [128, JS] ---------------------
    # t-form: t = (t + c)*g each step; trailing *g cancels in the ratio.
    td = T("td", [128, JS]); tn = T("tn", [128, JS])
    nc.vector.scalar_tensor_tensor(td[:, :], g[:, :], cden[:, 0:1], g[:, :], OP.mult, OP.bypass)
    nc.vector.scalar_tensor_tensor(tn[:, :], g[:, :], cnum[:, 0:1], g[:, :], OP.mult, OP.bypass)
    for k in range(1, NCOEF):
        nc.vector.scalar_tensor_tensor(td[:, :], td[:, :], cden[:, k:k + 1], g[:, :], OP.add, OP.mult)
        nc.vector.scalar_tensor_tensor(tn[:, :], tn[:, :], cnum[:, k:k + 1], g[:, :], OP.add, OP.mult)

    # ---------------- m = Num/Den, partial sum ------------------------------
    rden = T("rden", [128, JS])
    nc.vector.reciprocal(rden[:, :], td[:, :])
    mjunk = T("mjunk", [128, JS])
    nc.vector.tensor_mul(mjunk[:, :], tn[:, :], rden[:, :])
    # full 2048-element reduce on gpsimd (partition+free axes in one op)
    ms_sb = T("ms_sb", [1, 1])
    nc.gpsimd.tensor_reduce(ms_sb[:, :], mjunk[:, :], axis=AX.XYZWC, op=OP.add)

    # out = sv_sum/(4*S) * msum + bv_sum / (4*ncores)
    out_sb = T("out_sb", [1, 1])
    nc.vector.scalar_tensor_tensor(out_sb[:, :], ms_sb[:, :], svs[0:1, 0:1], bvt[:, :], OP.mult, OP.add)
    nc.sync.dma_start(out=d["out"].ap(), in_=out_sb[:, :])


def build_nc():
    nc = bacc.Bacc("TRN2", target_bir_lowering=False, debug=False,
                   enable_asserts=False, num_devices=NCORES)
    d = {}
    d["data"] = nc.dram_tensor("data", [129, 129], F32, kind="ExternalInput")
    d["EP"] = nc.dram_tensor("EP", [128, 37], F32, kind="ExternalInput")
    d["out"] = nc.dram_tensor("out", [1, 1], F32, kind="ExternalOutput")
    with tile.TileContext(nc) as tc:
        with ExitStack() as ctx:
            _emit(ctx, tc, d)
    nc.compile()
    return nc


_NC = None


def _get_nc():
    global _NC
    if _NC is None:
        _NC = build_nc()
    return _NC


def make_in_maps(inputs):
    data = np.ascontiguousarray(inputs["data"], np.float32)
    cw = np.asarray(inputs["conv_w"], np.float32).reshape(2, 2)
    cb = float(np.asarray(inputs["conv_b"], np.float32).reshape(()))
    Wq = np.asarray(inputs["Wq"], np.float32)
    bq = np.asarray(inputs["bq"], np.float32)
    Wk = np.asarray(inputs["Wk"], np.float32)
    Wv = np.asarray(inputs["Wv"], np.float32)
    bv = np.asarray(inputs["bv"], np.float32)

    pk = np.zeros((4, 21), np.float32)
    pk[:, 0:4] = Wk
    pk[:, 4] = 1.0
    pk[:, 5:9] = Wq
    pk[:, 9] = bq
    pk[:, 10:14] = Wv
    pk[:, 14] = bv
    pk[:, 15:19] = np.diag([cw[0, 0], cw[0, 1], cw[1, 0], cw[1, 1]]).astype(np.float32)
    pk[0, 19] = cb / 2.0
    pk[0, 20] = cb

    in_maps = []
    for c in range(NCORES):
        ep = np.zeros((128, 37), np.float32)
        ep[16 * c + np.arange(JS), np.arange(JS)] = 1.0
        ep[0:4, 16:37] = pk
        in_maps.append({"data": data, "EP": ep})
    return in_maps


def run_on_hw(inputs, trace=False, **kw):
    nc = _get_nc()
    res = run_bass_kernel_spmd(nc, make_in_maps(inputs),
                               core_ids=list(range(NCORES)), trace=trace, **kw)
    total = np.float64(0.0)
    for r in res.results:
        total += np.float64(r["out"][0, 0])
    return np.float32(total), res


def kernel(**inputs) -> np.ndarray:
    out, _ = run_on_hw(inputs, trace=False)
    return out


# revision 20
# speedup vs baseline: 1.0263x; 1.0263x over previous
"""Trainium2 Bass kernel for nn_ConvAttentionHybrid.

Math: the reference broadcasts the conv-sigmoid output f[s] along the embed
dim E, so q/k/v are affine (rank-1) in f.  The softmax logits collapse to
    l[s,t] = g[s]*f[t] + (terms constant in t),   g[s] = 0.5*(A*f[s] + C)
with A = rowsum(Wq).rowsum(Wk), C = bq.rowsum(Wk).  With h = f - 1/2:
    m(s) = Num(s)/Den(s)
    Den(s) = sum_n g^n/n! * W_n,          W_n = sum_t h_t^n
    Num(s) = sum_n g^n/n! * (W_{n+1} + W_n/2)
(the common e^{g/2} factor cancels in the ratio), and
    result = sv_sum*sum_s m(s)/(4*S) + bv_sum/4.
|g| <= ~1.1 and |h| <= 1/2 here, so 7 Taylor terms are exact to ~3e-6,
far below the 2e-2 gate.  Each core computes f and the moments fully
(cheap) and evaluates m(s) for its own 2048-element s-chunk, fed by a
host-sliced [128,34] window of `data` (so no one-hot select matmul);
the host sums the 8 partial outputs.

Perf structure vs the first version:
  - all params packed into ONE [4,20] DMA; A/C/sv_sum/bv_sum come from
    one tiny PE matmul; conv weights broadcast to 128 partitions via PE
    (no 128-line broadcast DMAs in front of the data DMAs)
  - dataA on sync queue, dataB issued from the tensor engine queue so
    the two 66KB loads run on different rings
  - conv split vector/gpsimd (2+2 shifted MACs), sigmoid accumulates W_1
  - moments: W_2/W_4 as Square-activations with accum on scalar, W_3/5/6/7
    as fused tensor_tensor_reduce on vector, running concurrently
  - Horner: Den chain on gpsimd, Num chain on vector, 7 coeffs each
"""

import math
from contextlib import ExitStack

import numpy as np

import concourse.bass as bass
import concourse.tile as tile
from concourse import bacc, mybir
from concourse.bass_utils import run_bass_kernel_spmd

AF = mybir.ActivationFunctionType
OP = mybir.AluOpType
AX = mybir.AxisListType
F32 = mybir.dt.float32

NCORES = 8
NCOEF = 4             # Taylor coefficients n = 0..NCOEF-1
NMOM = NCOEF + 1      # moments W_0 .. W_NCOEF
JS = 16               # s-chunk columns per core (128*16 = 2048 s per core)
S_TOTAL = 16384


def _emit(ctx: ExitStack, tc: "tile.TileContext", d):
    nc = tc.nc
    pool = ctx.enter_context(tc.tile_pool(name="main", bufs=1))
    psum = ctx.enter_context(tc.tile_pool(name="ps", bufs=1, space="PSUM"))

    def T(name, shape):
        return pool.tile(shape, F32, tag=name, name=name)

    def P(name, shape):
        return psum.tile(shape, F32, tag=name, name=name)

    # ---------------- DMAs: dataA alone on sync (earliest issue) ------------
    dataA = T("dataA", [128, 129])
    nc.sync.dma_start(out=dataA[:, :], in_=d["data"].ap()[0:128, :])
    dataB = T("dataB", [128, 129])
    nc.scalar.dma_start(out=dataB[:, 0:65], in_=d["data"].ap()[1:129, 0:65])
    pk = T("pk", [4, 21])
    nc.gpsimd.dma_start(out=pk[:, :], in_=d["pk"].ap())
    nc.gpsimd.dma_start(out=dataB[:, 65:129], in_=d["data"].ap()[1:129, 65:129])
    e_sb = T("e_sb", [128, JS])
    nc.gpsimd.dma_start(out=e_sb[:, :], in_=d["E"].ap())

    # ---------------- early constants (vector + gpsimd, in DMA shadow) -----
    ones4x128 = T("ones4x128", [4, 128])
    halfrow = T("halfrow", [1, 128])
    ones4c = T("ones4c", [4, 1])
    wacc = T("wacc", [128, NMOM])
    nc.vector.memset(ones4x128[:, :], 1.0)
    nc.vector.memset(halfrow[:, :], 0.5)
    nc.vector.memset(ones4c[:, :], 1.0)
    nc.vector.memset(wacc[:, NMOM - 1:NMOM], 128.0)    # W_0 partial (=128*128)

    ones128 = T("ones128", [128, 128])
    invfT = T("invfT", [128, NCOEF])
    invfT2 = T("invfT2", [128, NCOEF])
    nc.gpsimd.memset(ones128[:, :], 1.0)
    for k in range(NCOEF):
        n = NCOEF - 1 - k
        nc.gpsimd.memset(invfT[:, k:k + 1], 1.0 / (math.factorial(n) * 2.0 ** n))
        nc.gpsimd.memset(invfT2[:, k:k + 1], 1.0 / (math.factorial(n) * 2.0 ** (n + 1)))

    # dummy activations: trigger act table loads while DMAs are in flight
    dums = T("dums", [4, 1])
    nc.scalar.activation(dums[:, :], ones4x128[0:4, 0:1], AF.Sigmoid, bias=0.0, scale=1.0)
    nc.scalar.activation(dums[:, :], ones4x128[0:4, 0:1], AF.Tanh, bias=0.0, scale=1.0)
    nc.scalar.activation(dums[:, :], ones4x128[0:4, 0:1], AF.Square, bias=0.0, scale=1.0)

    # ---------------- PE: param matmuls (only need pk) ----------------------
    # conv-weight broadcast: wb[128, 0:4] = w00,w01,w10,w11; col 4 = conv_b
    wb_ps = P("wb_ps", [128, 6])
    nc.tensor.matmul(wb_ps[:, :], ones4x128[:, :], pk[:, 15:21], start=True, stop=True)

    # vector: wb to SBUF; row-sum columns on partitions 0:4
    wbs = T("wbs", [128, 6])
    nc.vector.tensor_copy(wbs[:, :], wb_ps[:, :])
    akc = T("akc", [4, 1])
    rhs4 = T("rhs4", [4, 4])
    nc.vector.reduce_sum(akc[:, :], pk[:, 0:4], axis=AX.X)
    nc.vector.reduce_sum(rhs4[:, 0:1], pk[:, 5:9], axis=AX.X)
    nc.vector.tensor_copy(rhs4[:, 1:2], pk[:, 9:10])
    nc.vector.reduce_sum(rhs4[:, 2:3], pk[:, 10:14], axis=AX.X)
    nc.vector.tensor_copy(rhs4[:, 3:4], pk[:, 14:15])

    # PE: [A, C] = ak^T @ [aq|bq],  [sv, bvs] = ones^T @ [svc|bvc]
    ac_ps = P("ac_ps", [1, 2])
    nc.tensor.matmul(ac_ps[:, :], akc[:, :], rhs4[:, 0:2], start=True, stop=True)
    sb_ps = P("sb_ps", [1, 2])
    nc.tensor.matmul(sb_ps[:, :], ones4c[:, :], rhs4[:, 2:4], start=True, stop=True)

    # ---------------- conv + sigmoid -> f [128,128] ------------------------
    # u1 on scalar (Copy with per-partition scale), u2..u4 chained on vector
    u1 = T("u1", [128, 128]); u2 = T("u2", [128, 128])
    u3 = T("u3", [128, 128]); u4 = T("u4", [128, 128])
    h = T("h", [128, 128]); h2 = T("h2", [128, 128])
    h3 = T("h3", [128, 128])
    junk = T("junk", [128, 128])
    with tc.high_priority():
        nc.scalar.activation(u1[:, :], dataA[:, 0:128], AF.Copy, bias=0.0, scale=wbs[:, 0:1])
        nc.vector.scalar_tensor_tensor(u2[:, :], dataA[:, 1:129], wbs[:, 1:2], u1[:, :], OP.mult, OP.add)
        nc.vector.scalar_tensor_tensor(u3[:, :], dataB[:, 0:128], wbs[:, 2:3], u2[:, :], OP.mult, OP.add)
        nc.vector.scalar_tensor_tensor(u4[:, :], dataB[:, 1:129], wbs[:, 3:4], u3[:, :], OP.mult, OP.add)
        # t = tanh(0.5*conv + 0.5*cb) = 2h  (W_n = W'_n/2^n folded into
        # invfT/invfT2); W'_1 on vector so scalar goes straight to squares
        nc.scalar.activation(h[:, :], u4[:, :], AF.Tanh, bias=wbs[:, 4:5], scale=0.5)

        # ---------------- moments W'_n = sum t^n ----------------------------
        # wacc col j holds the per-partition partial of W'_{NMOM-1-j}
        nc.scalar.activation(h2[:, :], h[:, :], AF.Square, bias=0.0, scale=1.0,
                             accum_out=wacc[:, NMOM - 3:NMOM - 2])
        nc.scalar.activation(junk[:, :], h2[:, :], AF.Square, bias=0.0, scale=1.0,
                             accum_out=wacc[:, NMOM - 5:NMOM - 4])
        nc.vector.reduce_sum(wacc[:, NMOM - 2:NMOM - 1], h[:, :], axis=AX.X)
        nc.vector.tensor_mul(h3[:, :], h2[:, :], h[:, :])
        nc.vector.reduce_sum(wacc[:, NMOM - 4:NMOM - 3], h3[:, :], axis=AX.X)

    # ---------------- per-core chunk: one-hot row select of conv(u4) --------
    # sel[m,j] = u4[16c+j, m] (transposed chunk; sum over s is order-free)
    sel_ps = P("sel_ps", [128, JS])
    nc.tensor.matmul(sel_ps[:, :], u4[:, :], e_sb[:, :], start=True, stop=True)

    # scalar: chunk sigmoid + g = halfA*fc + halfC (needs pbc from PE chain)
    bc_ps = P("bc_ps", [128, 2])
    ac_sb = T("ac_sb", [1, 2])
    sb_sb = T("sb_sb", [1, 2])
    nc.vector.tensor_copy(ac_sb[:, :], ac_ps[:, :])
    nc.vector.tensor_copy(sb_sb[:, :], sb_ps[:, :])
    nc.tensor.matmul(bc_ps[:, :], halfrow[:, :], ac_sb[0:1, 0:2], start=True, stop=True)
    pbc = T("pbc", [128, 2])
    nc.vector.tensor_copy(pbc[:, :], bc_ps[:, :])
    bvt = T("bvt", [1, 1])
    nc.vector.tensor_scalar_mul(bvt[:, :], sb_sb[0:1, 1:2], 1.0 / (4.0 * NCORES))
    svs = T("svs", [1, 1])
    nc.vector.tensor_scalar_mul(svs[:, :], sb_sb[0:1, 0:1], 1.0 / (4.0 * S_TOTAL))

    fc = T("fc", [128, JS])
    ga = T("ga", [128, JS])
    g = T("g", [128, JS])
    nc.scalar.activation(fc[:, :], sel_ps[:, :], AF.Sigmoid, bias=wbs[:, 5:6], scale=1.0)
    nc.vector.scalar_tensor_tensor(ga[:, :], fc[:, :], pbc[:, 0:1], fc[:, :], OP.mult, OP.bypass)
    nc.vector.scalar_tensor_tensor(g[:, :], ga[:, :], pbc[:, 1:2], ga[:, :], OP.add, OP.bypass)

    # ---------------- W totals broadcast to all partitions ------------------
    wrow_ps = P("wrow_ps", [128, NMOM])
    nc.tensor.matmul(wrow_ps[:, :], ones128[:, :], wacc[:, 0:NMOM], start=True, stop=True)

    # ---------------- Taylor coefficients (Horner order, all partitions) ----
    # cden[k] = W_{N-1-k}/(N-1-k)!,  cnum[k] = 0.5*cden[k] + W_{N-k}/(N-1-k)!
    cden = T("cden", [128, NCOEF])
    cdsh = T("cdsh", [128, NCOEF])
    cnum = T("cnum", [128, NCOEF])
    nc.vector.tensor_mul(cden[:, :], wrow_ps[:, 1:NMOM], invfT[:, :])
    nc.vector.tensor_mul(cdsh[:, :], wrow_ps[:, 0:NCOEF], invfT2[:, :])
    nc.vector.scalar_tensor_tensor(cnum[:, :], cden[:, :], 0.5, cdsh[:, :], OP.mult, OP.add)

    # ---------------- fused Den/Num Horner on [128, JS] ---------------------
    # t-form: t = (t + c)*g each step; trailing *g cancels in the ratio.
    td = T("td", [128, JS]); tn = T("tn", [128, JS])
    nc.vector.scalar_tensor_tensor(td[:, :], g[:, :], cden[:, 0:1], g[:, :], OP.mult, OP.bypass)
    nc.vector.scalar_tensor_tensor(tn[:, :], g[:, :], cnum[:, 0:1], g[:, :], OP.mult, OP.bypass)
    for k in range(1, NCOEF):
        nc.vector.scalar_tensor_tensor(td[:, :], td[:, :], cden[:, k:k + 1], g[:, :], OP.add, OP.mult)
        nc.vector.scalar_tensor_tensor(tn[:, :], tn[:, :], cnum[:, k:k + 1], g[:, :], OP.add, OP.mult)

    # ---------------- m = Num/Den, partial sum ------------------------------
    rden = T("rden", [128, JS])
    nc.vector.reciprocal(rden[:, :], td[:, :])
    mjunk = T("mjunk", [128, JS])
    nc.vector.tensor_mul(mjunk[:, :], tn[:, :], rden[:, :])
    # full 2048-element reduce on gpsimd (partition+free axes in one op)
    ms_sb = T("ms_sb", [1, 1])
    nc.gpsimd.tensor_reduce(ms_sb[:, :], mjunk[:, :], axis=AX.XYZWC, op=OP.add)

    # out = sv_sum/(4*S) * msum + bv_sum / (4*ncores)
    out_sb = T("out_sb", [1, 1])
    nc.vector.scalar_tensor_tensor(out_sb[:, :], ms_sb[:, :], svs[0:1, 0:1], bvt[:, :], OP.mult, OP.add)
    nc.sync.dma_start(out=d["out"].ap(), in_=out_sb[:, :])


def build_nc():
    nc = bacc.Bacc("TRN2", target_bir_lowering=False, debug=False,
                   enable_asserts=False, num_devices=NCORES)
    d = {}
    d["data"] = nc.dram_tensor("data", [129, 129], F32, kind="ExternalInput")
    d["E"] = nc.dram_tensor("E", [128, JS], F32, kind="ExternalInput")
    d["pk"] = nc.dram_tensor("pk", [4, 21], F32, kind="ExternalInput")
    d["out"] = nc.dram_tensor("out", [1, 1], F32, kind="ExternalOutput")
    with tile.TileContext(nc) as tc:
        with ExitStack() as ctx:
            _emit(ctx, tc, d)
    nc.compile()
    return nc


_NC = None


def _get_nc():
    global _NC
    if _NC is None:
        _NC = build_nc()
    return _NC


def make_in_maps(inputs):
    data = np.ascontiguousarray(inputs["data"], np.float32)
    cw = np.asarray(inputs["conv_w"], np.float32).reshape(2, 2)
    cb = float(np.asarray(inputs["conv_b"], np.float32).reshape(()))
    Wq = np.asarray(inputs["Wq"], np.float32)
    bq = np.asarray(inputs["bq"], np.float32)
    Wk = np.asarray(inputs["Wk"], np.float32)
    Wv = np.asarray(inputs["Wv"], np.float32)
    bv = np.asarray(inputs["bv"], np.float32)

    pk = np.zeros((4, 21), np.float32)
    pk[:, 0:4] = Wk
    pk[:, 4] = 1.0
    pk[:, 5:9] = Wq
    pk[:, 9] = bq
    pk[:, 10:14] = Wv
    pk[:, 14] = bv
    pk[:, 15:19] = np.diag([cw[0, 0], cw[0, 1], cw[1, 0], cw[1, 1]]).astype(np.float32)
    pk[0, 19] = cb / 2.0
    pk[0, 20] = cb

    in_maps = []
    for c in range(NCORES):
        e = np.zeros((128, JS), np.float32)
        e[16 * c + np.arange(JS), np.arange(JS)] = 1.0
        in_maps.append({"data": data, "E": e, "pk": pk})
    return in_maps


def run_on_hw(inputs, trace=False, **kw):
    nc = _get_nc()
    res = run_bass_kernel_spmd(nc, make_in_maps(inputs),
                               core_ids=list(range(NCORES)), trace=trace, **kw)
    total = np.float64(0.0)
    for r in res.results:
        total += np.float64(r["out"][0, 0])
    return np.float32(total), res


def kernel(**inputs) -> np.ndarray:
    out, _ = run_on_hw(inputs, trace=False)
    return out
